# revision 17
# baseline (speedup 1.0000x reference)
"""Causal self-attention on 8 TRN2 NeuronCores (Bass/Tile, fp32r).

Sharding: core c = 4*bp + hg handles batches [2bp, 2bp+1] and heads
[4hg, 4hg+4). Host transposes x to [B, D, S], slices weights per head
group, sums the 4 head-group partial outputs per batch pair.

Per-core kernel (per batch):
  A) QKV projection from xT tiles: qT/kT in [head_dim, token] layout
     (matmul lhsT = w slice), v directly in [token, head_dim] layout
     (matmul lhsT = xT tile, rhs = w_v) with a ones column appended.
  B) Flash-style causal attention per head: scores^T blocks [j,i] on PE,
     exp (with 1/sqrt(hd) scale folded in) on ACT, mask on diagonal
     blocks on DVE, AV accumulation on PE; ones column of V yields the
     softmax denominators in psum row 64; normalize via reciprocal +
     partition-broadcast DMA + DVE multiply.
  C) Output projection (partial: only this core's head rows of w_proj).
"""
import numpy as np

B, S, D, H = 4, 2048, 1024, 16
HD = D // H            # 64
SCALE = 1.0 / np.sqrt(HD)
NB = 2                 # batches per core
NHC = 4                # heads per core
HCOLS = NHC * HD       # 256 q/k/v columns per core
NDT = D // 128         # 8 D-tiles
NJT = S // 128         # 16 j-tiles per batch
NIT = S // 512         # 4 i-tiles per batch

_NC = None
LAST_RESULT = None


def _build():
    import concourse.bacc as bacc
    import concourse.mybir as mybir
    import concourse.tile as tile

    f32 = mybir.dt.float32
    f32r = mybir.dt.float32r
    Act = mybir.ActivationFunctionType

    nc = bacc.Bacc(trn_type="TRN2", target_bir_lowering=False)
    xT = nc.dram_tensor("xT", [NB, D, S], f32, kind="ExternalInput")
    wqk = nc.dram_tensor("wqk", [D, 2 * HCOLS], f32, kind="ExternalInput")
    wv = nc.dram_tensor("wv", [D, HCOLS], f32, kind="ExternalInput")
    wo = nc.dram_tensor("wo", [HCOLS, D], f32, kind="ExternalInput")
    bqk = nc.dram_tensor("bqk", [2 * HCOLS], f32, kind="ExternalInput")
    bv = nc.dram_tensor("bv", [HCOLS], f32, kind="ExternalInput")
    mask = nc.dram_tensor("mask", [128, 4 * 512], f32, kind="ExternalInput")
    y = nc.dram_tensor("y", [NB, S, D], f32, kind="ExternalOutput")

    import concourse.bass as bass

    with tile.TileContext(nc) as tc:
        with (
            tc.tile_pool(name="singles", bufs=1) as singles,
            tc.tile_pool(name="xtp", bufs=2) as xtp,
            tc.tile_pool(name="qkp", bufs=1) as qkp,
            tc.tile_pool(name="vp", bufs=1) as vp,
            tc.tile_pool(name="attp", bufs=8) as attp,
            tc.tile_pool(name="yhp", bufs=2) as yhp,
            tc.tile_pool(name="smalls", bufs=4) as smalls,
            tc.tile_pool(name="outp", bufs=2) as outp,
            tc.tile_pool(name="dscr", bufs=4, space="DRAM") as dscrp,
            tc.tile_pool(name="psA", bufs=2, space="PSUM") as psA,
            tc.tile_pool(name="psS", bufs=2, space="PSUM") as psS,
            tc.tile_pool(name="psY", bufs=4, space="PSUM") as psY,
        ):
            # ---- one-time loads (weights, biases, masks) ----
            wqk_sb = singles.tile([128, NDT, 2 * HCOLS], f32r, tag="wqk")
            nc.sync.dma_start(
                out=wqk_sb,
                in_=wqk.ap().bitcast(f32r).rearrange("(dt p) c -> p dt c", p=128),
            )
            wv_sb = singles.tile([128, NDT, HCOLS], f32r, tag="wv")
            nc.sync.dma_start(
                out=wv_sb,
                in_=wv.ap().bitcast(f32r).rearrange("(dt p) c -> p dt c", p=128),
            )
            wo_sb = singles.tile([128, 2, D], f32r, tag="wo")
            nc.sync.dma_start(
                out=wo_sb,
                in_=wo.ap().bitcast(f32r).rearrange("(kt p) c -> p kt c", p=128),
            )
            bqk_sb = singles.tile([128, 4], f32, tag="bqk")
            nc.sync.dma_start(
                out=bqk_sb, in_=bqk.ap().rearrange("(cb p) -> p cb", p=128)
            )
            # bv broadcast across partitions: [HCOLS] -> [128, HCOLS]
            bv_sb = singles.tile([128, HCOLS], f32, tag="bv")
            bv_ap = bv.ap()
            nc.gpsimd.dma_start(
                out=bv_sb,
                in_=bass.AP(
                    tensor=bv_ap.tensor, offset=bv_ap.offset,
                    ap=[[0, 128], *bv_ap.ap],
                ),
            )
            ones_sb = singles.tile([128, 64], f32, tag="ones")
            nc.vector.memset(ones_sb[:], 1.0)
            mask_sb = singles.tile([128, 4, 512], f32r, tag="mask")
            nc.sync.dma_start(
                out=mask_sb,
                in_=mask.ap().bitcast(f32r).rearrange("p (r i) -> p r i", r=4),
            )

            for b in range(NB):
                # ================= Phase A: QKV projection =================
                sA = nc.enter_named_scope(f"qkv{b}", False)
                # qT/kT: [2 tiles of 128 rows, S]; v: [128, jt, h, 65]
                qkT = [
                    qkp.tile([128, S], f32r, tag=f"qkT{g}", name=f"qkT{g}_{b}")
                    for g in range(4)
                ]
                # qkT[0..1] = q rows 0-255, qkT[2..3] = k rows 0-255
                v_sb = vp.tile([128, NJT, NHC, HD + 1], f32r, tag="v")
                nc.vector.tensor_copy(
                    v_sb[:, :, :, HD : HD + 1],
                    ones_sb[:].rearrange("p (a b c) -> p a b c", a=NJT, b=NHC),
                )

                for ch in range(4):  # 512-token chunks
                    t0 = ch * 512
                    xt = xtp.tile([128, NDT, 512], f32r, tag="xt")
                    for dt in range(NDT):
                        nc.sync.dma_start(
                            out=xt[:, dt, :],
                            in_=xT.ap().bitcast(f32r)[
                                b, dt * 128 : (dt + 1) * 128, t0 : t0 + 512
                            ],
                        )
                    for cb in range(4):  # q0 q1 k0 k1
                        ps = psA.tile([128, 512], f32, tag="ps")
                        for dt in range(NDT):
                            nc.tensor.matmul(
                                ps[:],
                                wqk_sb[:, dt, cb * 128 : (cb + 1) * 128],
                                xt[:, dt, :],
                                start=(dt == 0), stop=(dt == NDT - 1),
                            )
                        nc.scalar.activation(
                            out=qkT[cb][:, t0 : t0 + 512], in_=ps[:],
                            func=Act.Identity,
                            bias=bqk_sb[:, cb : cb + 1], scale=1.0,
                        )
                    for st in range(4):  # 128-token tiles within chunk
                        tok = t0 + st * 128
                        pool = psS if st % 2 else psA
                        vtag = "pss" if pool is psS else "ps"
                        psv = pool.tile(
                            [128, 512], f32, tag=vtag, name=f"psv_{b}_{tok}"
                        )
                        for dt in range(NDT):
                            nc.tensor.matmul(
                                psv[:, 0:HCOLS],
                                xt[:, dt, st * 128 : (st + 1) * 128],
                                wv_sb[:, dt, :],
                                start=(dt == 0), stop=(dt == NDT - 1),
                            )
                        nc.vector.tensor_add(
                            v_sb[:, tok // 128, :, 0:HD],
                            psv[:, 0:HCOLS].rearrange("p (h c) -> p h c", h=NHC),
                            bv_sb[:].rearrange("p (h c) -> p h c", h=NHC),
                        )

                nc.leave_named_scope(f"qkv{b}", sA[0], False)
                # ================= Phase B: causal attention =================
                sB = nc.enter_named_scope(f"attn{b}", False)
                yh = [
                    yhp.tile([128, S], f32r, tag=f"yh{g}", name=f"yh{g}_{b}")
                    for g in range(2)
                ]
                for h in range(NHC):
                    g, po = h // 2, 64 * (h % 2)
                    # jt-major: each kT j-tile / V j-tile weight is used by
                    # up to 4 consecutive matmuls (one per live i-tile), so
                    # the expensive fp32r weight load amortizes. All 4 AV
                    # accumulators stay live (psY bufs=4).
                    psy = [
                        psY.tile([HD + 1, 512], f32, tag="psy",
                                 name=f"psy_{b}_{h}_{it}")
                        for it in range(NIT)
                    ]
                    atts = {}
                    for jt in range(NJT):
                        it_lo = jt // 4
                        for it in range(it_lo, NIT):
                            pool = psS if it % 2 else psA
                            tag = "pss" if pool is psS else "ps"
                            pss = pool.tile(
                                [128, 512], f32, tag=tag,
                                name=f"pss_{b}_{h}_{it}_{jt}",
                            )
                            nc.tensor.matmul(
                                pss[:],
                                qkT[2 + g][po : po + 64,
                                           jt * 128 : (jt + 1) * 128],
                                qkT[g][po : po + 64,
                                       it * 512 : (it + 1) * 512],
                                start=True, stop=True,
                            )
                            att = attp.tile([128, 512], f32r, tag="att")
                            nc.scalar.activation(
                                out=att[:], in_=pss[:], func=Act.Exp,
                                bias=0.0, scale=float(SCALE),
                            )
                            if jt == 4 * it:  # diagonal block: causal mask
                                nc.gpsimd.tensor_mul(
                                    att[:], att[:], mask_sb[:, 0, :]
                                )
                            elif jt > 4 * it and jt - 4 * it < 4:
                                nc.gpsimd.tensor_mul(
                                    att[:], att[:], mask_sb[:, jt - 4 * it, :]
                                )
                            atts[it] = att
                        for it in range(it_lo, NIT):
                            nc.tensor.matmul(
                                psy[it][:],
                                v_sb[:, jt, h, :],
                                atts[it][:],
                                start=(jt == 0), stop=(jt == 4 * it + 3),
                            )
                    for it in range(NIT):
                        # stage AV result out of PSUM fast (frees the
                        # accumulator slot), then normalize from SBUF:
                        # bounce denom row via DRAM to broadcast across
                        # partitions, reciprocal + multiply on DVE.
                        stg = smalls.tile([HD + 1, 512], f32, tag="stg")
                        nc.scalar.copy(stg[:], psy[it][:])
                        dsc = dscrp.tile([1, 512], f32, tag="dsc")
                        nc.sync.dma_start(out=dsc, in_=stg[64:65, :])
                        bc = smalls.tile([64, 512], f32, tag="bc")
                        nc.gpsimd.dma_start(
                            out=bc[:],
                            in_=bass.AP(
                                tensor=dsc.tensor, offset=dsc.offset,
                                ap=[[0, 64], [1, 512]],
                            ),
                        )
                        nc.vector.reciprocal(bc[:], bc[:])
                        nc.vector.tensor_mul(
                            yh[g][po : po + 64, it * 512 : (it + 1) * 512],
                            stg[0:HD, :], bc[:],
                        )

                nc.leave_named_scope(f"attn{b}", sB[0], False)
                # ================= Phase C: output projection =================
                sC = nc.enter_named_scope(f"proj{b}", False)
                for tt2 in range(S // 128):
                    yo = outp.tile([128, D], f32, tag="yo")
                    for oc in range(2):
                        pool = psS if (tt2 + oc) % 2 else psA
                        ptag = "pss" if pool is psS else "ps"
                        pso = pool.tile(
                            [128, 512], f32, tag=ptag, name=f"pso_{b}_{tt2}_{oc}"
                        )
                        for kt in range(2):
                            nc.tensor.matmul(
                                pso[:],
                                yh[kt][:, tt2 * 128 : (tt2 + 1) * 128],
                                wo_sb[:, kt, oc * 512 : (oc + 1) * 512],
                                start=(kt == 0), stop=(kt == 1),
                            )
                        nc.vector.tensor_copy(yo[:, oc * 512 : (oc + 1) * 512], pso[:])
                    nc.sync.dma_start(
                        out=y.ap()[b, tt2 * 128 : (tt2 + 1) * 128, :], in_=yo[:]
                    )

                nc.leave_named_scope(f"proj{b}", sC[0], False)

    nc.compile()
    return nc


def _get_nc():
    global _NC
    if _NC is None:
        _NC = _build()
    return _NC


def kernel(x, w_qkv, b_qkv, w_proj, b_proj):
    global LAST_RESULT
    from concourse.bass_utils import run_bass_kernel_spmd

    x = np.asarray(x, dtype=np.float32)
    w_qkv = np.asarray(w_qkv, dtype=np.float32)
    b_qkv = np.asarray(b_qkv, dtype=np.float32)
    w_proj = np.asarray(w_proj, dtype=np.float32)
    b_proj = np.asarray(b_proj, dtype=np.float32)

    xTb = np.ascontiguousarray(x.transpose(0, 2, 1))  # [B, D, S]

    # causal masks for the 4 diagonal block offsets r: allow j'+128r <= i'
    jj = np.arange(128)[:, None]
    ii = np.arange(512)[None, :]
    mask = np.concatenate(
        [(jj + 128 * r <= ii).astype(np.float32) for r in range(4)], axis=1
    )  # [128, 2048]

    in_maps = []
    for c in range(8):
        bp, hg = c // 4, c % 4
        cols = slice(hg * HCOLS, (hg + 1) * HCOLS)
        w_q = w_qkv[:, cols]
        w_k = w_qkv[:, D : 2 * D][:, cols]
        w_v = w_qkv[:, 2 * D : 3 * D][:, cols]
        in_maps.append({
            "xT": np.ascontiguousarray(xTb[2 * bp : 2 * bp + 2]),
            "wqk": np.ascontiguousarray(np.concatenate([w_q, w_k], axis=1)),
            "wv": np.ascontiguousarray(w_v),
            "wo": np.ascontiguousarray(w_proj[cols, :]),
            "bqk": np.ascontiguousarray(
                np.concatenate([b_qkv[cols], b_qkv[D : 2 * D][cols]])
            ),
            "bv": np.ascontiguousarray(b_qkv[2 * D : 3 * D][cols]),
            "mask": mask,
        })

    nc = _get_nc()
    res = run_bass_kernel_spmd(nc, in_maps, core_ids=list(range(8)))
    LAST_RESULT = res

    out = np.zeros((B, S, D), dtype=np.float32)
    for c in range(8):
        bp = c // 4
        out[2 * bp : 2 * bp + 2] += res.results[c]["y"]
    out += b_proj[None, None, :]
    return out


# revision 19
# speedup vs baseline: 1.0855x; 1.0855x over previous
"""Causal self-attention on 8 TRN2 NeuronCores (Bass/Tile, fp32r).

Sharding: core c = 4*bp + hg handles batches [2bp, 2bp+1] and heads
[4hg, 4hg+4). Host transposes x to [B, D, S], slices weights per head
group, sums the 4 head-group partial outputs per batch pair.

Per-core kernel (per batch):
  A) QKV projection from xT tiles: qT/kT in [head_dim, token] layout
     (matmul lhsT = w slice), v directly in [token, head_dim] layout
     (matmul lhsT = xT tile, rhs = w_v) with a ones column appended.
  B) Flash-style causal attention per head: scores^T blocks [j,i] on PE,
     exp (with 1/sqrt(hd) scale folded in) on ACT, mask on diagonal
     blocks on DVE, AV accumulation on PE; ones column of V yields the
     softmax denominators in psum row 64; normalize via reciprocal +
     partition-broadcast DMA + DVE multiply.
  C) Output projection (partial: only this core's head rows of w_proj).
"""
import numpy as np

B, S, D, H = 4, 2048, 1024, 16
HD = D // H            # 64
SCALE = 1.0 / np.sqrt(HD)
NB = 2                 # batches per core
NHC = 4                # heads per core
HCOLS = NHC * HD       # 256 q/k/v columns per core
NDT = D // 128         # 8 D-tiles
NJT = S // 128         # 16 j-tiles per batch
NIT = S // 512         # 4 i-tiles per batch

_NC = None
LAST_RESULT = None


def _build():
    import concourse.bacc as bacc
    import concourse.mybir as mybir
    import concourse.tile as tile

    f32 = mybir.dt.float32
    f32r = mybir.dt.float32r
    Act = mybir.ActivationFunctionType

    nc = bacc.Bacc(trn_type="TRN2", target_bir_lowering=False)
    xT = nc.dram_tensor("xT", [NB, D, S], f32, kind="ExternalInput")
    wqk = nc.dram_tensor("wqk", [D, 2 * HCOLS], f32, kind="ExternalInput")
    wv = nc.dram_tensor("wv", [D, HCOLS], f32, kind="ExternalInput")
    wo = nc.dram_tensor("wo", [HCOLS, D], f32, kind="ExternalInput")
    bqk = nc.dram_tensor("bqk", [2 * HCOLS], f32, kind="ExternalInput")
    bv = nc.dram_tensor("bv", [HCOLS], f32, kind="ExternalInput")
    mask = nc.dram_tensor("mask", [128, 4 * 512], f32, kind="ExternalInput")
    y = nc.dram_tensor("y", [NB, S, D], f32, kind="ExternalOutput")

    import concourse.bass as bass

    with tile.TileContext(nc) as tc:
        with (
            tc.tile_pool(name="singles", bufs=1) as singles,
            tc.tile_pool(name="xtp", bufs=2) as xtp,
            tc.tile_pool(name="qkp", bufs=1) as qkp,
            tc.tile_pool(name="vp", bufs=1) as vp,
            tc.tile_pool(name="attp", bufs=6) as attp,
            tc.tile_pool(name="yhp", bufs=2) as yhp,
            tc.tile_pool(name="smalls", bufs=3) as smalls,
            tc.tile_pool(name="outp", bufs=2) as outp,
            tc.tile_pool(name="dscr", bufs=4, space="DRAM") as dscrp,
            tc.tile_pool(name="psA", bufs=2, space="PSUM") as psA,
            tc.tile_pool(name="psS", bufs=2, space="PSUM") as psS,
            tc.tile_pool(name="psY", bufs=4, space="PSUM") as psY,
        ):
            # ---- one-time loads (weights, biases, masks) ----
            wqk_sb = singles.tile([128, NDT, 2 * HCOLS], f32r, tag="wqk")
            nc.sync.dma_start(
                out=wqk_sb,
                in_=wqk.ap().bitcast(f32r).rearrange("(dt p) c -> p dt c", p=128),
            )
            wv_sb = singles.tile([128, NDT, HCOLS], f32r, tag="wv")
            nc.sync.dma_start(
                out=wv_sb,
                in_=wv.ap().bitcast(f32r).rearrange("(dt p) c -> p dt c", p=128),
            )
            wo_sb = singles.tile([128, 2, D], f32r, tag="wo")
            nc.sync.dma_start(
                out=wo_sb,
                in_=wo.ap().bitcast(f32r).rearrange("(kt p) c -> p kt c", p=128),
            )
            bqk_sb = singles.tile([128, 4], f32, tag="bqk")
            nc.sync.dma_start(
                out=bqk_sb, in_=bqk.ap().rearrange("(cb p) -> p cb", p=128)
            )
            # bv broadcast across partitions: [HCOLS] -> [128, HCOLS]
            bv_sb = singles.tile([128, HCOLS], f32, tag="bv")
            bv_ap = bv.ap()
            nc.gpsimd.dma_start(
                out=bv_sb,
                in_=bass.AP(
                    tensor=bv_ap.tensor, offset=bv_ap.offset,
                    ap=[[0, 128], *bv_ap.ap],
                ),
            )
            ones_sb = singles.tile([128, 64], f32, tag="ones")
            nc.vector.memset(ones_sb[:], 1.0)
            zeros_sb = singles.tile([128, 1024], f32, tag="zeros")
            nc.vector.memset(zeros_sb[:], 0.0)
            mask_sb = singles.tile([128, 4, 512], f32r, tag="mask")
            nc.sync.dma_start(
                out=mask_sb,
                in_=mask.ap().bitcast(f32r).rearrange("p (r i) -> p r i", r=4),
            )

            for b in range(NB):
                # ================= Phase A: QKV projection =================
                sA = nc.enter_named_scope(f"qkv{b}", False)
                # q: per-head zero-padded [128, S] tiles (rows 64*(h%2) hold
                # q_h, other 64 rows zero) so scores matmuls run K=128 with the
                # packed kT tile as weights -- K<128 matmuls never warm the PE
                # clock gate. k: packed [2 tiles of 128 rows, S].
                qp = [
                    qkp.tile([128, S], f32r, tag=f"qp{h}", name=f"qp{h}_{b}")
                    for h in range(NHC)
                ]
                kT = [
                    qkp.tile([128, S], f32r, tag=f"kT{g}", name=f"kT{g}_{b}")
                    for g in range(2)
                ]
                if b == 0:  # zero the pad halves once (bufs=1 slots persist)
                    for h in range(NHC):
                        zo = 64 * ((h + 1) % 2)
                        for half in range(2):
                            nc.vector.tensor_copy(
                                qp[h][zo : zo + 64,
                                      half * 1024 : (half + 1) * 1024],
                                zeros_sb[0:64, :],
                            )
                v_sb = vp.tile([128, NJT, NHC, HD + 1], f32r, tag="v")
                nc.vector.tensor_copy(
                    v_sb[:, :, :, HD : HD + 1],
                    ones_sb[:].rearrange("p (a b c) -> p a b c", a=NJT, b=NHC),
                )

                for ch in range(4):  # 512-token chunks
                    t0 = ch * 512
                    xt = xtp.tile([128, NDT, 512], f32r, tag="xt")
                    for dt in range(NDT):
                        nc.sync.dma_start(
                            out=xt[:, dt, :],
                            in_=xT.ap().bitcast(f32r)[
                                b, dt * 128 : (dt + 1) * 128, t0 : t0 + 512
                            ],
                        )
                    for cb in range(4):  # q0 q1 k0 k1
                        ps = psA.tile([128, 512], f32, tag="ps")
                        for dt in range(NDT):
                            nc.tensor.matmul(
                                ps[:],
                                wqk_sb[:, dt, cb * 128 : (cb + 1) * 128],
                                xt[:, dt, :],
                                start=(dt == 0), stop=(dt == NDT - 1),
                            )
                        if cb < 2:  # q tiles: split halves into padded tiles
                            for hh in range(2):
                                po2 = 64 * hh
                                nc.scalar.activation(
                                    out=qp[2 * cb + hh][po2 : po2 + 64,
                                                        t0 : t0 + 512],
                                    in_=ps[po2 : po2 + 64, :],
                                    func=Act.Identity,
                                    bias=bqk_sb[po2 : po2 + 64, cb : cb + 1],
                                    scale=1.0,
                                )
                        else:
                            nc.scalar.activation(
                                out=kT[cb - 2][:, t0 : t0 + 512], in_=ps[:],
                                func=Act.Identity,
                                bias=bqk_sb[:, cb : cb + 1], scale=1.0,
                            )
                    for st in range(4):  # 128-token tiles within chunk
                        tok = t0 + st * 128
                        pool = psS if st % 2 else psA
                        vtag = "pss" if pool is psS else "ps"
                        psv = pool.tile(
                            [128, 512], f32, tag=vtag, name=f"psv_{b}_{tok}"
                        )
                        for dt in range(NDT):
                            nc.tensor.matmul(
                                psv[:, 0:HCOLS],
                                xt[:, dt, st * 128 : (st + 1) * 128],
                                wv_sb[:, dt, :],
                                start=(dt == 0), stop=(dt == NDT - 1),
                            )
                        nc.vector.tensor_add(
                            v_sb[:, tok // 128, :, 0:HD],
                            psv[:, 0:HCOLS].rearrange("p (h c) -> p h c", h=NHC),
                            bv_sb[:].rearrange("p (h c) -> p h c", h=NHC),
                        )

                nc.leave_named_scope(f"qkv{b}", sA[0], False)
                # ================= Phase B: causal attention =================
                sB = nc.enter_named_scope(f"attn{b}", False)
                yh = [
                    yhp.tile([128, S], f32r, tag=f"yh{g}", name=f"yh{g}_{b}")
                    for g in range(2)
                ]
                for h in range(NHC):
                    g, po = h // 2, 64 * (h % 2)
                    # jt-major: each kT j-tile / V j-tile weight is used by
                    # up to 4 consecutive matmuls (one per live i-tile), so
                    # the expensive fp32r weight load amortizes. All 4 AV
                    # accumulators stay live (psY bufs=4).
                    psy = [
                        psY.tile([HD + 1, 512], f32, tag="psy",
                                 name=f"psy_{b}_{h}_{it}")
                        for it in range(NIT)
                    ]
                    atts = {}
                    for jt in range(NJT):
                        it_lo = jt // 4
                        for it in range(it_lo, NIT):
                            pool = psS if it % 2 else psA
                            tag = "pss" if pool is psS else "ps"
                            pss = pool.tile(
                                [128, 512], f32, tag=tag,
                                name=f"pss_{b}_{h}_{it}_{jt}",
                            )
                            nc.tensor.matmul(
                                pss[:],
                                kT[g][:, jt * 128 : (jt + 1) * 128],
                                qp[h][:, it * 512 : (it + 1) * 512],
                                start=True, stop=True,
                            )
                            att = attp.tile([128, 512], f32r, tag="att")
                            nc.scalar.activation(
                                out=att[:], in_=pss[:], func=Act.Exp,
                                bias=0.0, scale=float(SCALE),
                            )
                            if jt == 4 * it:  # diagonal block: causal mask
                                nc.gpsimd.tensor_mul(
                                    att[:], att[:], mask_sb[:, 0, :]
                                )
                            elif jt > 4 * it and jt - 4 * it < 4:
                                nc.gpsimd.tensor_mul(
                                    att[:], att[:], mask_sb[:, jt - 4 * it, :]
                                )
                            atts[it] = att
                        for it in range(it_lo, NIT):
                            nc.tensor.matmul(
                                psy[it][:],
                                v_sb[:, jt, h, :],
                                atts[it][:],
                                start=(jt == 0), stop=(jt == 4 * it + 3),
                            )
                    for it in range(NIT):
                        # stage AV result out of PSUM fast (frees the
                        # accumulator slot), then normalize from SBUF:
                        # bounce denom row via DRAM to broadcast across
                        # partitions, reciprocal + multiply on DVE.
                        stg = smalls.tile([HD + 1, 512], f32, tag="stg")
                        nc.scalar.copy(stg[:], psy[it][:])
                        dsc = dscrp.tile([1, 512], f32, tag="dsc")
                        nc.sync.dma_start(out=dsc, in_=stg[64:65, :])
                        bc = smalls.tile([64, 512], f32, tag="bc")
                        nc.gpsimd.dma_start(
                            out=bc[:],
                            in_=bass.AP(
                                tensor=dsc.tensor, offset=dsc.offset,
                                ap=[[0, 64], [1, 512]],
                            ),
                        )
                        nc.vector.reciprocal(bc[:], bc[:])
                        nc.vector.tensor_mul(
                            yh[g][po : po + 64, it * 512 : (it + 1) * 512],
                            stg[0:HD, :], bc[:],
                        )

                nc.leave_named_scope(f"attn{b}", sB[0], False)
                # ================= Phase C: output projection =================
                sC = nc.enter_named_scope(f"proj{b}", False)
                for tt2 in range(S // 128):
                    yo = outp.tile([128, D], f32, tag="yo")
                    for oc in range(2):
                        pool = psS if (tt2 + oc) % 2 else psA
                        ptag = "pss" if pool is psS else "ps"
                        pso = pool.tile(
                            [128, 512], f32, tag=ptag, name=f"pso_{b}_{tt2}_{oc}"
                        )
                        for kt in range(2):
                            nc.tensor.matmul(
                                pso[:],
                                yh[kt][:, tt2 * 128 : (tt2 + 1) * 128],
                                wo_sb[:, kt, oc * 512 : (oc + 1) * 512],
                                start=(kt == 0), stop=(kt == 1),
                            )
                        nc.vector.tensor_copy(yo[:, oc * 512 : (oc + 1) * 512], pso[:])
                    nc.sync.dma_start(
                        out=y.ap()[b, tt2 * 128 : (tt2 + 1) * 128, :], in_=yo[:]
                    )

                nc.leave_named_scope(f"proj{b}", sC[0], False)

    nc.compile()
    return nc


def _get_nc():
    global _NC
    if _NC is None:
        _NC = _build()
    return _NC


def kernel(x, w_qkv, b_qkv, w_proj, b_proj):
    global LAST_RESULT
    from concourse.bass_utils import run_bass_kernel_spmd

    x = np.asarray(x, dtype=np.float32)
    w_qkv = np.asarray(w_qkv, dtype=np.float32)
    b_qkv = np.asarray(b_qkv, dtype=np.float32)
    w_proj = np.asarray(w_proj, dtype=np.float32)
    b_proj = np.asarray(b_proj, dtype=np.float32)

    xTb = np.ascontiguousarray(x.transpose(0, 2, 1))  # [B, D, S]

    # causal masks for the 4 diagonal block offsets r: allow j'+128r <= i'
    jj = np.arange(128)[:, None]
    ii = np.arange(512)[None, :]
    mask = np.concatenate(
        [(jj + 128 * r <= ii).astype(np.float32) for r in range(4)], axis=1
    )  # [128, 2048]

    in_maps = []
    for c in range(8):
        bp, hg = c // 4, c % 4
        cols = slice(hg * HCOLS, (hg + 1) * HCOLS)
        w_q = w_qkv[:, cols]
        w_k = w_qkv[:, D : 2 * D][:, cols]
        w_v = w_qkv[:, 2 * D : 3 * D][:, cols]
        in_maps.append({
            "xT": np.ascontiguousarray(xTb[2 * bp : 2 * bp + 2]),
            "wqk": np.ascontiguousarray(np.concatenate([w_q, w_k], axis=1)),
            "wv": np.ascontiguousarray(w_v),
            "wo": np.ascontiguousarray(w_proj[cols, :]),
            "bqk": np.ascontiguousarray(
                np.concatenate([b_qkv[cols], b_qkv[D : 2 * D][cols]])
            ),
            "bv": np.ascontiguousarray(b_qkv[2 * D : 3 * D][cols]),
            "mask": mask,
        })

    nc = _get_nc()
    res = run_bass_kernel_spmd(nc, in_maps, core_ids=list(range(8)))
    LAST_RESULT = res

    out = np.zeros((B, S, D), dtype=np.float32)
    for c in range(8):
        bp = c // 4
        out[2 * bp : 2 * bp + 2] += res.results[c]["y"]
    out += b_proj[None, None, :]
    return out


# revision 20
# speedup vs baseline: 1.2673x; 1.1675x over previous
"""Causal self-attention on 8 TRN2 NeuronCores (Bass/Tile, fp32r).

Sharding: core c = 4*bp + hg handles batches [2bp, 2bp+1] and heads
[4hg, 4hg+4). Host transposes x to [B, D, S], slices weights per head
group, sums the 4 head-group partial outputs per batch pair.

Per-core kernel (per batch):
  A) QKV projection from xT tiles: qT/kT in [head_dim, token] layout
     (matmul lhsT = w slice), v directly in [token, head_dim] layout
     (matmul lhsT = xT tile, rhs = w_v) with a ones column appended.
  B) Flash-style causal attention per head: scores^T blocks [j,i] on PE,
     exp (with 1/sqrt(hd) scale folded in) on ACT, mask on diagonal
     blocks on DVE, AV accumulation on PE; ones column of V yields the
     softmax denominators in psum row 64; normalize via reciprocal +
     partition-broadcast DMA + DVE multiply.
  C) Output projection (partial: only this core's head rows of w_proj).
"""
import numpy as np

B, S, D, H = 4, 2048, 1024, 16
HD = D // H            # 64
SCALE = 1.0 / np.sqrt(HD)
NB = 2                 # batches per core
NHC = 4                # heads per core
HCOLS = NHC * HD       # 256 q/k/v columns per core
NDT = D // 128         # 8 D-tiles
NJT = S // 128         # 16 j-tiles per batch
NIT = S // 512         # 4 i-tiles per batch

_NC = None
LAST_RESULT = None


def _build():
    import concourse.bacc as bacc
    import concourse.mybir as mybir
    import concourse.tile as tile

    f32 = mybir.dt.float32
    f32r = mybir.dt.float32r
    Act = mybir.ActivationFunctionType

    nc = bacc.Bacc(trn_type="TRN2", target_bir_lowering=False)
    xT = nc.dram_tensor("xT", [NB, D, S], f32, kind="ExternalInput")
    wqk = nc.dram_tensor("wqk", [D, 2 * HCOLS], f32, kind="ExternalInput")
    wv = nc.dram_tensor("wv", [D, HCOLS], f32, kind="ExternalInput")
    wo = nc.dram_tensor("wo", [HCOLS, D], f32, kind="ExternalInput")
    bqk = nc.dram_tensor("bqk", [2 * HCOLS], f32, kind="ExternalInput")
    bv = nc.dram_tensor("bv", [HCOLS], f32, kind="ExternalInput")
    mask = nc.dram_tensor("mask", [128, 4 * 512], f32, kind="ExternalInput")
    y = nc.dram_tensor("y", [NB, S, D], f32, kind="ExternalOutput")

    import concourse.bass as bass

    with tile.TileContext(nc) as tc:
        with (
            tc.tile_pool(name="singles", bufs=1) as singles,
            tc.tile_pool(name="xtp", bufs=2) as xtp,
            tc.tile_pool(name="qkp", bufs=1) as qkp,
            tc.tile_pool(name="vp", bufs=1) as vp,
            tc.tile_pool(name="attp", bufs=6) as attp,
            tc.tile_pool(name="yhp", bufs=2) as yhp,
            tc.tile_pool(name="smalls", bufs=3) as smalls,
            tc.tile_pool(name="outp", bufs=2) as outp,
            tc.tile_pool(name="dscr", bufs=4, space="DRAM") as dscrp,
            tc.tile_pool(name="psA", bufs=2, space="PSUM") as psA,
            tc.tile_pool(name="psS", bufs=2, space="PSUM") as psS,
            tc.tile_pool(name="psY", bufs=4, space="PSUM") as psY,
        ):
            # ---- one-time loads (weights, biases, masks) ----
            wqk_sb = singles.tile([128, NDT, 2 * HCOLS], f32r, tag="wqk")
            nc.sync.dma_start(
                out=wqk_sb,
                in_=wqk.ap().bitcast(f32r).rearrange("(dt p) c -> p dt c", p=128),
            )
            wv_sb = singles.tile([128, NDT, HCOLS], f32r, tag="wv")
            nc.sync.dma_start(
                out=wv_sb,
                in_=wv.ap().bitcast(f32r).rearrange("(dt p) c -> p dt c", p=128),
            )
            wo_sb = singles.tile([128, 2, D], f32r, tag="wo")
            nc.sync.dma_start(
                out=wo_sb,
                in_=wo.ap().bitcast(f32r).rearrange("(kt p) c -> p kt c", p=128),
            )
            bqk_sb = singles.tile([128, 4], f32, tag="bqk")
            nc.sync.dma_start(
                out=bqk_sb, in_=bqk.ap().rearrange("(cb p) -> p cb", p=128)
            )
            # bv broadcast across partitions: [HCOLS] -> [128, HCOLS]
            bv_sb = singles.tile([128, HCOLS], f32, tag="bv")
            bv_ap = bv.ap()
            nc.gpsimd.dma_start(
                out=bv_sb,
                in_=bass.AP(
                    tensor=bv_ap.tensor, offset=bv_ap.offset,
                    ap=[[0, 128], *bv_ap.ap],
                ),
            )
            ones_sb = singles.tile([128, 64], f32, tag="ones")
            nc.vector.memset(ones_sb[:], 1.0)
            zeros_sb = singles.tile([128, 1024], f32, tag="zeros")
            nc.vector.memset(zeros_sb[:], 0.0)
            mask_sb = singles.tile([128, 4, 512], f32r, tag="mask")
            nc.sync.dma_start(
                out=mask_sb,
                in_=mask.ap().bitcast(f32r).rearrange("p (r i) -> p r i", r=4),
            )

            for b in range(NB):
                # ================= Phase A: QKV projection =================
                sA = nc.enter_named_scope(f"qkv{b}", False)
                # q: per-head zero-padded [128, S] tiles (rows 64*(h%2) hold
                # q_h, other 64 rows zero) so scores matmuls run K=128 with the
                # packed kT tile as weights -- K<128 matmuls never warm the PE
                # clock gate. k: packed [2 tiles of 128 rows, S].
                qp = [
                    qkp.tile([128, S], f32r, tag=f"qp{h}", name=f"qp{h}_{b}")
                    for h in range(NHC)
                ]
                kT = [
                    qkp.tile([128, S], f32r, tag=f"kT{g}", name=f"kT{g}_{b}")
                    for g in range(2)
                ]
                if b == 0:  # zero the pad halves once (bufs=1 slots persist)
                    for h in range(NHC):
                        zo = 64 * ((h + 1) % 2)
                        for half in range(2):
                            nc.vector.tensor_copy(
                                qp[h][zo : zo + 64,
                                      half * 1024 : (half + 1) * 1024],
                                zeros_sb[0:64, :],
                            )
                v_sb = vp.tile([128, NJT, NHC, HD + 1], f32r, tag="v")
                nc.vector.tensor_copy(
                    v_sb[:, :, :, HD : HD + 1],
                    ones_sb[:].rearrange("p (a b c) -> p a b c", a=NJT, b=NHC),
                )

                for ch in range(4):  # 512-token chunks
                    t0 = ch * 512
                    xt = xtp.tile([128, NDT, 512], f32r, tag="xt")
                    for dt in range(NDT):
                        nc.sync.dma_start(
                            out=xt[:, dt, :],
                            in_=xT.ap().bitcast(f32r)[
                                b, dt * 128 : (dt + 1) * 128, t0 : t0 + 512
                            ],
                        )
                    for cb in range(4):  # q0 q1 k0 k1
                        ps = psA.tile([128, 512], f32, tag="ps")
                        for dt in range(NDT):
                            nc.tensor.matmul(
                                ps[:],
                                wqk_sb[:, dt, cb * 128 : (cb + 1) * 128],
                                xt[:, dt, :],
                                start=(dt == 0), stop=(dt == NDT - 1),
                            )
                        if cb < 2:  # q tiles: split halves into padded tiles
                            for hh in range(2):
                                po2 = 64 * hh
                                nc.scalar.activation(
                                    out=qp[2 * cb + hh][po2 : po2 + 64,
                                                        t0 : t0 + 512],
                                    in_=ps[po2 : po2 + 64, :],
                                    func=Act.Identity,
                                    bias=bqk_sb[po2 : po2 + 64, cb : cb + 1],
                                    scale=1.0,
                                )
                        else:
                            nc.scalar.activation(
                                out=kT[cb - 2][:, t0 : t0 + 512], in_=ps[:],
                                func=Act.Identity,
                                bias=bqk_sb[:, cb : cb + 1], scale=1.0,
                            )
                    for st in range(4):  # 128-token tiles within chunk
                        tok = t0 + st * 128
                        pool = psS if st % 2 else psA
                        vtag = "pss" if pool is psS else "ps"
                        psv = pool.tile(
                            [128, 512], f32, tag=vtag, name=f"psv_{b}_{tok}"
                        )
                        for dt in range(NDT):
                            nc.tensor.matmul(
                                psv[:, 0:HCOLS],
                                xt[:, dt, st * 128 : (st + 1) * 128],
                                wv_sb[:, dt, :],
                                start=(dt == 0), stop=(dt == NDT - 1),
                            )
                        nc.vector.tensor_add(
                            v_sb[:, tok // 128, :, 0:HD],
                            psv[:, 0:HCOLS].rearrange("p (h c) -> p h c", h=NHC),
                            bv_sb[:].rearrange("p (h c) -> p h c", h=NHC),
                        )

                nc.leave_named_scope(f"qkv{b}", sA[0], False)
                # ================= Phase B: causal attention =================
                sB = nc.enter_named_scope(f"attn{b}", False)
                yh = [
                    yhp.tile([128, S], f32r, tag=f"yh{g}", name=f"yh{g}_{b}")
                    for g in range(2)
                ]
                for h in range(NHC):
                    g, po = h // 2, 64 * (h % 2)
                    # jt-major: each kT j-tile / V j-tile weight is used by
                    # up to 4 consecutive matmuls (one per live i-tile), so
                    # the expensive fp32r weight load amortizes. All 4 AV
                    # accumulators stay live (psY bufs=4).
                    psy = [
                        psY.tile([HD + 1, 512], f32, tag="psy",
                                 name=f"psy_{b}_{h}_{it}")
                        for it in range(NIT)
                    ]
                    atts = {}
                    for jt in range(NJT):
                        it_lo = jt // 4
                        for it in range(it_lo, NIT):
                            pool = psS if it % 2 else psA
                            tag = "pss" if pool is psS else "ps"
                            pss = pool.tile(
                                [128, 512], f32, tag=tag,
                                name=f"pss_{b}_{h}_{it}_{jt}",
                            )
                            nc.tensor.matmul(
                                pss[:],
                                kT[g][:, jt * 128 : (jt + 1) * 128],
                                qp[h][:, it * 512 : (it + 1) * 512],
                                start=True, stop=True,
                            )
                            att = attp.tile([128, 512], f32r, tag="att")
                            nc.scalar.activation(
                                out=att[:], in_=pss[:], func=Act.Exp,
                                bias=0.0, scale=float(SCALE),
                            )
                            if it == it_lo and jt - 4 * it < 4:
                                # diagonal block: causal mask on DVE
                                nc.vector.tensor_mul(
                                    att[:], att[:], mask_sb[:, jt - 4 * it, :]
                                )
                            atts[it] = att
                        for it in range(it_lo, NIT):
                            nc.tensor.matmul(
                                psy[it][:],
                                v_sb[:, jt, h, :],
                                atts[it][:],
                                start=(jt == 0), stop=(jt == 4 * it + 3),
                            )
                    for it in range(NIT):
                        # stage AV result out of PSUM fast (frees the
                        # accumulator slot), then normalize from SBUF:
                        # bounce denom row via DRAM to broadcast across
                        # partitions, reciprocal + multiply on DVE.
                        stg = smalls.tile([HD + 1, 512], f32, tag="stg")
                        nc.scalar.copy(stg[:], psy[it][:])
                        dsc = dscrp.tile([1, 512], f32, tag="dsc")
                        nc.sync.dma_start(out=dsc, in_=stg[64:65, :])
                        bc = smalls.tile([64, 512], f32, tag="bc")
                        nc.gpsimd.dma_start(
                            out=bc[:],
                            in_=bass.AP(
                                tensor=dsc.tensor, offset=dsc.offset,
                                ap=[[0, 64], [1, 512]],
                            ),
                        )
                        nc.vector.reciprocal(bc[:], bc[:])
                        nc.vector.tensor_mul(
                            yh[g][po : po + 64, it * 512 : (it + 1) * 512],
                            stg[0:HD, :], bc[:],
                        )

                nc.leave_named_scope(f"attn{b}", sB[0], False)
                # ================= Phase C: output projection =================
                sC = nc.enter_named_scope(f"proj{b}", False)
                for tt2 in range(S // 128):
                    yo = outp.tile([128, D], f32, tag="yo")
                    for oc in range(2):
                        pool = psS if (tt2 + oc) % 2 else psA
                        ptag = "pss" if pool is psS else "ps"
                        pso = pool.tile(
                            [128, 512], f32, tag=ptag, name=f"pso_{b}_{tt2}_{oc}"
                        )
                        for kt in range(2):
                            nc.tensor.matmul(
                                pso[:],
                                yh[kt][:, tt2 * 128 : (tt2 + 1) * 128],
                                wo_sb[:, kt, oc * 512 : (oc + 1) * 512],
                                start=(kt == 0), stop=(kt == 1),
                            )
                        nc.vector.tensor_copy(yo[:, oc * 512 : (oc + 1) * 512], pso[:])
                    nc.sync.dma_start(
                        out=y.ap()[b, tt2 * 128 : (tt2 + 1) * 128, :], in_=yo[:]
                    )

                nc.leave_named_scope(f"proj{b}", sC[0], False)

    nc.compile()
    return nc


def _get_nc():
    global _NC
    if _NC is None:
        _NC = _build()
    return _NC


def kernel(x, w_qkv, b_qkv, w_proj, b_proj):
    global LAST_RESULT
    from concourse.bass_utils import run_bass_kernel_spmd

    x = np.asarray(x, dtype=np.float32)
    w_qkv = np.asarray(w_qkv, dtype=np.float32)
    b_qkv = np.asarray(b_qkv, dtype=np.float32)
    w_proj = np.asarray(w_proj, dtype=np.float32)
    b_proj = np.asarray(b_proj, dtype=np.float32)

    xTb = np.ascontiguousarray(x.transpose(0, 2, 1))  # [B, D, S]

    # causal masks for the 4 diagonal block offsets r: allow j'+128r <= i'
    jj = np.arange(128)[:, None]
    ii = np.arange(512)[None, :]
    mask = np.concatenate(
        [(jj + 128 * r <= ii).astype(np.float32) for r in range(4)], axis=1
    )  # [128, 2048]

    in_maps = []
    for c in range(8):
        bp, hg = c // 4, c % 4
        cols = slice(hg * HCOLS, (hg + 1) * HCOLS)
        w_q = w_qkv[:, cols]
        w_k = w_qkv[:, D : 2 * D][:, cols]
        w_v = w_qkv[:, 2 * D : 3 * D][:, cols]
        in_maps.append({
            "xT": np.ascontiguousarray(xTb[2 * bp : 2 * bp + 2]),
            "wqk": np.ascontiguousarray(np.concatenate([w_q, w_k], axis=1)),
            "wv": np.ascontiguousarray(w_v),
            "wo": np.ascontiguousarray(w_proj[cols, :]),
            "bqk": np.ascontiguousarray(
                np.concatenate([b_qkv[cols], b_qkv[D : 2 * D][cols]])
            ),
            "bv": np.ascontiguousarray(b_qkv[2 * D : 3 * D][cols]),
            "mask": mask,
        })

    nc = _get_nc()
    res = run_bass_kernel_spmd(nc, in_maps, core_ids=list(range(8)))
    LAST_RESULT = res

    out = np.zeros((B, S, D), dtype=np.float32)
    for c in range(8):
        bp = c // 4
        out[2 * bp : 2 * bp + 2] += res.results[c]["y"]
    out += b_proj[None, None, :]
    return out
